# revision 19
# baseline (speedup 1.0000x reference)
"""Trainium-2 Bass kernel for nn_EnhancedGNNEncoder (4-layer bipartite GNN).

8 NeuronCores, one SPMD Bass program. Nodes canonically sharded; edges
sharded by destination owner with a per-core-uniform chunk schedule
(64-node dst windows x 25k-row src banks). Message rows fetched with
dma_gather (4 SWDGE queues); segment-sum via ON-CHIP generated one-hot
matrices (iota==dloc)*ew in bf16, scattered feature-major on the tensor
engine (lhsT = gathered messages f32r, rhs = one-hot bf16) into [64,512]
PSUM groups; node states hT kept SBUF-resident; degree reciprocals
precomputed once per direction and staged in DRAM rows; per-pass message
tables exchanged with AllGather collectives.
"""
import numpy as np
import ml_dtypes

import concourse.bacc as bacc
import concourse.mybir as mybir
from concourse.tile import TileContext
from concourse.bass_utils import run_bass_kernel_spmd
from concourse._compat import get_trn_type, cdiv
from concourse.library_config import mlp as mlp_lib
from concourse.masks import make_identity

BF16 = ml_dtypes.bfloat16
NCORES = 8
WIN = 64
GRP = 8          # windows per update group (512 dst cols)
BANK = 25000
GCALL = 16
D = 64
LN_EPS = 1e-3
LAYERS = 4
NV, NC_, NK = 100000, 50000, 20000
EF = 8
AF = mybir.ActivationFunctionType
ALU = mybir.AluOpType
AX = mybir.AxisListType

DIRS = ["vc0", "vc1", "vk0", "vk1"]
SRCT = {"vc0": "var", "vc1": "cons", "vk0": "var", "vk1": "cut"}
DSTT = {"vc0": "cons", "vc1": "var", "vk0": "cut", "vk1": "var"}
NTYPE = {"var": NV, "cons": NC_, "cut": NK}
FD = {"var": 19, "cons": 5, "cut": 30}
TSRC = {"vc0": "tvar_vc", "vc1": "tcons_vc", "vk0": "tvar_vk", "vk1": "tcut_vk"}
TDST = {"vc0": "tcons_vc", "vc1": "tvar_vc", "vk0": "tcut_vk", "vk1": "tvar_vk"}


def _wrap_idx(flat):
    n = len(flat)
    a = np.zeros((16, cdiv(n, 16)), dtype=np.int16)
    a[np.arange(n) % 16, np.arange(n) // 16] = flat
    return np.tile(a, (8, 1))


class DirSchedule:
    """Uniform schedule for one direction. Chunk streams are stored in
    *bank-stream* column order: col(k) = bank_base[bank(k)] + pos_in_bank(k)."""

    def __init__(self, src, dst, ef, n_src, n_dst, name):
        self.name, self.n_src, self.n_dst = name, n_src, n_dst
        self.shard = n_dst // NCORES
        self.nwin = cdiv(self.shard, WIN)
        self.nbank = cdiv(n_src, BANK)
        owner = dst // self.shard
        dloc = dst - owner * self.shard
        w_of = dloc // WIN
        b_of = src // BANK

        per = {}
        cpb = np.ones(self.nbank, dtype=np.int64)
        for c in range(NCORES):
            m = np.flatnonzero(owner == c)
            key = w_of[m].astype(np.int64) * self.nbank + b_of[m]
            e = m[np.argsort(key, kind="stable")]
            per[c] = e
            for bk in range(self.nbank):
                sel = e[b_of[e] == bk]
                if sel.size:
                    cnt = np.bincount(w_of[sel], minlength=self.nwin)
                    cpb[bk] = max(cpb[bk], cdiv(int(cnt.max()), 128))
        self.cpb = [int(x) for x in cpb]
        self.cpw = sum(self.cpb)
        self.nchunk = self.nwin * self.cpw
        slot_bank = []
        for bk in range(self.nbank):
            slot_bank += [bk] * self.cpb[bk]
        self.slot_bank = slot_bank
        # bank-stream maps
        self.bank_nchunk = [self.nwin * self.cpb[bk] for bk in range(self.nbank)]
        self.bank_base = np.concatenate([[0], np.cumsum(self.bank_nchunk)])[:-1]
        self.colmap = np.zeros(self.nchunk, dtype=np.int64)
        pos = [0] * self.nbank
        for k in range(self.nchunk):
            bk = slot_bank[k % self.cpw]
            self.colmap[k] = self.bank_base[bk] + pos[bk]
            pos[bk] += 1

        # per-core: dloc table (within-window dst index, -1 for empty slots),
        # edge-feature pack, src/dst gather indices
        self.dlocT = np.full((NCORES, 128, self.nchunk), -1.0, dtype=np.float32)
        self.efp = np.zeros((NCORES, EF + 1, self.nchunk * 128), dtype=np.float32)
        self.efp[:, EF, :] = 1.0
        self.src_idx, self.dst_idx = [], []
        for c in range(NCORES):
            e = per[c]
            sflat = np.zeros(self.nchunk * 128, dtype=np.int16)
            dflat = np.zeros(self.nchunk * 128, dtype=np.int16)
            for wv in range(self.nwin):
                base = wv * self.cpw
                off = 0
                we = e[w_of[e] == wv]
                for bk in range(self.nbank):
                    sel = we[b_of[we] == bk]
                    for j in range(self.cpb[bk]):
                        k = base + off + j
                        col = self.colmap[k]
                        part = sel[j * 128:(j + 1) * 128]
                        n = len(part)
                        if n:
                            self.dlocT[c, :n, col] = (
                                dloc[part] - wv * WIN).astype(np.float32)
                            self.efp[c, :EF, col * 128:col * 128 + n] = ef[part].T
                            sflat[col * 128:col * 128 + n] = (
                                src[part] - bk * BANK).astype(np.int16)
                            dflat[col * 128:col * 128 + n] = dloc[part].astype(np.int16)
                    off += self.cpb[bk]
            self.src_idx.append(_wrap_idx(sflat))
            self.dst_idx.append(_wrap_idx(dflat))
        self.idx_cols = cdiv(self.nchunk * 128, 16)


def build_schedules(inputs):
    vc, vk = inputs["var_cons_edges"], inputs["var_cut_edges"]
    efvc, efvk = inputs["var_cons_edge_features"], inputs["var_cut_edge_features"]
    return {
        "vc0": DirSchedule(vc[0], vc[1], efvc, NV, NC_, "vc0"),
        "vc1": DirSchedule(vc[1], vc[0], efvc, NC_, NV, "vc1"),
        "vk0": DirSchedule(vk[0], vk[1], efvk, NV, NK, "vk0"),
        "vk1": DirSchedule(vk[1], vk[0], efvk, NK, NV, "vk1"),
    }


def build_kernel(scheds):
    f32, bf16, i16 = mybir.dt.float32, mybir.dt.bfloat16, mybir.dt.int16
    f32r = mybir.dt.float32r
    nc = bacc.Bacc(get_trn_type() or "TRN2", num_swdge_queues=4)
    RG = [list(range(NCORES))]
    ins = {}

    def I(name, shape, dt=f32):
        ins[name] = nc.dram_tensor(name, shape, dt, kind="ExternalInput")
        return ins[name]

    for t in ("var", "cons", "cut"):
        I(f"featT_{t}", [FD[t] + 1, NTYPE[t] // NCORES])
        I(f"emb_{t}_w1", [FD[t] + 1, D])
        I(f"emb_{t}_w2", [D, D])
        I(f"emb_{t}_b2", [1, D])
    for d in DIRS:
        s = scheds[d]
        I(f"sidx_{d}", [128, s.idx_cols], i16)
        I(f"didx_{d}", [128, s.idx_cols], i16)
        I(f"efT_{d}", [EF + 1, s.nchunk * 128])
        I(f"dlocT_{d}", [128, s.nchunk])
    I("iotaB", [128, WIN])
    for et in ("vc", "vk"):
        I(f"ew_{et}_w1v", [D, D])
        I(f"ew_{et}_w1o", [D, D])
        I(f"ew_{et}_w1e", [EF + 1, D])
        I(f"ew_{et}_w2t", [1, 8 * D])
        I(f"ew_{et}_b2", [1, 1])
    I("mp_msg_w1", [16, D, D]); I("mp_msg_b1", [16, 1, D])
    I("mp_msg_w2", [16, D, D]); I("mp_msg_b2", [16, 1, D])
    I("mp_gate_w", [16, 2 * D, D]); I("mp_gate_b", [16, 1, D])
    I("mp_upd_w1", [16, 2 * D, D]); I("mp_upd_b1", [16, 1, D])
    I("mp_upd_w2", [16, D, D]); I("mp_upd_b2", [16, 1, D])
    I("mp_ln_g", [16, 1, D]); I("mp_ln_b", [16, 1, D])

    out_t = nc.dram_tensor("out_cut", [NK // NCORES, D], f32, kind="ExternalOutput")

    tcc_in, tcc_out = {}, {}
    for nm, t in (("tvar_vc", "var"), ("tcons_vc", "cons"),
                  ("tvar_vk", "var"), ("tcut_vk", "cut")):
        tcc_in[nm] = nc.dram_tensor(f"cci_{nm}", [NTYPE[t] // NCORES, D], f32)
        tcc_out[nm] = nc.dram_tensor(f"cco_{nm}", [NTYPE[t], D], f32,
                                     addr_space="Shared")
    msg_in, msg_out = {}, {}
    for l in range(LAYERS):
        for d in DIRS:
            msg_in[(l, d)] = nc.dram_tensor(
                f"mci{l}{d}", [NTYPE[SRCT[d]] // NCORES, D], f32)
            msg_out[(l, d)] = nc.dram_tensor(
                f"mco{l}{d}", [NTYPE[SRCT[d]], D], f32, addr_space="Shared")
    recip_d = {d: nc.dram_tensor(f"recip_{d}", [1, scheds[d].shard], f32)
               for d in DIRS}

    with TileContext(nc) as tc:
        with (
            tc.tile_pool(name="const", bufs=1) as cp,
            tc.tile_pool(name="sb", bufs=2) as sb,
            tc.tile_pool(name="mgp", bufs=5) as mgp,
            tc.tile_pool(name="wtp", bufs=8) as wtp,
            tc.tile_pool(name="ps", bufs=2, space="PSUM") as ps,
            tc.tile_pool(name="psw", bufs=2, space="PSUM") as psw,
            tc.tile_pool(name="pst", bufs=1, space="PSUM") as pst,
            tc.tile_pool(name="psd", bufs=1, space="PSUM") as psd,
        ):
            nc.gpsimd.load_library(mlp_lib)
            ident = cp.tile([128, 128], f32)
            make_identity(nc, ident)
            ones1 = cp.tile([1, 128], f32)
            nc.gpsimd.memset(ones1[:], 1.0)
            iotaB = cp.tile([128, WIN], f32)
            nc.sync.dma_start(iotaB[:], ins["iotaB"][:])
            ones128c = cp.tile([128, 1], f32)
            nc.gpsimd.memset(ones128c[:], 1.0)
            onesD = cp.tile([1, D], f32)
            nc.gpsimd.memset(onesD[:], 1.0)
            invDc = cp.tile([D, 1], f32)
            nc.gpsimd.memset(invDc[:], 1.0 / D)

            def mmul(o, lt, r, st=True, sp=True):
                nc.tensor.matmul(o, lt, r, start=st, stop=sp)

            def col_of(row_dram, n, tag="colc", pool=None):
                rr = sb.tile([1, 128], f32, tag="colr")
                nc.sync.dma_start(rr[:, :n], row_dram)
                p = pst.tile([128, 128], f32, tag="tr")
                nc.tensor.transpose(p[:n, :1], rr[:1, :n], ident[:1, :1])
                c = (pool or sb).tile([128, 1], f32, tag=tag)
                nc.scalar.copy(c[:n, :], p[:n, :1])
                return c

            # ---------------- resident node states ----------------
            hT = {t: cp.tile([D, NTYPE[t] // NCORES], f32, tag=f"hT{t}",
                             name=f"hT_{t}")
                  for t in ("var", "cons", "cut")}

            # ---------------- preload per-pass bias columns ----------------
            mpb = {}
            for nm in ("mp_msg_b1", "mp_msg_b2", "mp_gate_b", "mp_upd_b1",
                       "mp_upd_b2", "mp_ln_g", "mp_ln_b"):
                t_ = cp.tile([D, 16], f32, tag=nm)
                for i in range(16):
                    c = col_of(ins[nm][i], D, tag="prec")
                    nc.scalar.copy(t_[:, i:i + 1], c[:D, :])
                mpb[nm] = t_

            # ---------------- P1: embeddings -> resident hT ----------------
            for t in ("var", "cons", "cut"):
                w1 = sb.tile([FD[t] + 1, D], f32, tag="ew1")
                nc.sync.dma_start(w1[:], ins[f"emb_{t}_w1"][:])
                w2 = sb.tile([D, D], f32, tag="ew2")
                nc.sync.dma_start(w2[:], ins[f"emb_{t}_w2"][:])
                b2c = col_of(ins[f"emb_{t}_b2"][:, :], D)
                sh = NTYPE[t] // NCORES
                for g0 in range(0, sh, 512):
                    gw = min(512, sh - g0)
                    ft = sb.tile([FD[t] + 1, 512], f32, tag="combT")
                    nc.sync.dma_start(ft[:, :gw], ins[f"featT_{t}"][:, g0:g0 + gw])
                    p1 = ps.tile([128, 512], f32, tag="mm")
                    mmul(p1[:D, :gw], w1[:], ft[:, :gw])
                    r1 = sb.tile([D, 512], f32, tag="r1")
                    nc.scalar.activation(r1[:, :gw], p1[:D, :gw], AF.Relu)
                    p2 = ps.tile([128, 512], f32, tag="mm")
                    mmul(p2[:D, :gw], w2[:], r1[:, :gw])
                    nc.vector.tensor_scalar_add(hT[t][:, g0:g0 + gw],
                                                p2[:D, :gw], b2c[:D, :])

            # ---------------- P2a: t tables + AllGather ----------------
            for nm, t, wn in (("tvar_vc", "var", "ew_vc_w1v"),
                              ("tcons_vc", "cons", "ew_vc_w1o"),
                              ("tvar_vk", "var", "ew_vk_w1v"),
                              ("tcut_vk", "cut", "ew_vk_w1o")):
                wt = sb.tile([D, D], f32, tag="ew2")
                nc.sync.dma_start(wt[:], ins[wn][:])
                sh = NTYPE[t] // NCORES
                for g0 in range(0, sh, 512):
                    gw = min(512, sh - g0)
                    pt = ps.tile([128, 512], f32, tag="mm")
                    mmul(pt[:D, :gw], wt[:], hT[t][:, g0:g0 + gw])
                    tt = sb.tile([D, 512], f32, tag="r1")
                    nc.vector.tensor_copy(tt[:, :gw], pt[:D, :gw])
                    for j0 in range(0, gw, 128):
                        jw = min(128, gw - j0)
                        pn = pst.tile([128, 128], f32, tag="tr")
                        nc.tensor.transpose(pn[:jw, :D], tt[:, j0:j0 + jw], ident[:D, :D])
                        nb = sb.tile([128, D], f32, tag="nb")
                        nc.scalar.copy(nb[:jw, :], pn[:jw, :D])
                        nc.sync.dma_start(tcc_in[nm][g0 + j0:g0 + j0 + jw, :],
                                          nb[:jw, :])
                nc.gpsimd.collective_compute(
                    "AllGather", ALU.bypass, ins=[tcc_in[nm][:]],
                    outs=[tcc_out[nm][:]], replica_groups=RG)

            # ---------------- P2b: edge weights ----------------
            dloc_sb, ew_tiles = {}, {}
            for d in DIRS:
                s = scheds[d]
                et = "vc" if d[:2] == "vc" else "vk"
                dl = cp.tile([128, s.nchunk], f32, tag=f"dl{d}")
                nc.sync.dma_start(dl[:], ins[f"dlocT_{d}"][:])
                dloc_sb[d] = dl
                w1e = sb.tile([EF + 1, D], f32, tag="w1e")
                nc.sync.dma_start(w1e[:], ins[f"ew_{et}_w1e"][:])
                w2b = cp.tile([128, 8, D], f32, tag=f"w2b{et}")
                w2r = sb.tile([1, 8 * D], f32, tag="w2r")
                nc.sync.dma_start(w2r[:], ins[f"ew_{et}_w2t"][:])
                pw = ps.tile([128, 512], f32, tag="mm")
                mmul(pw[:], ones1[:], w2r[:])
                nc.vector.tensor_copy(
                    w2b[:].rearrange("p a b -> p (a b)")[:], pw[:])
                b2b = cp.tile([128, 1], f32, tag=f"eb2{et}")
                pb2 = ps.tile([128, 512], f32, tag="mm")
                b2r = sb.tile([1, 1], f32, tag="b2r1")
                nc.sync.dma_start(b2r[:], ins[f"ew_{et}_b2"][:])
                mmul(pb2[:, :1], ones1[:], b2r[:])
                nc.vector.tensor_copy(b2b[:], pb2[:, :1])

                ew_sb = cp.tile([128, s.nchunk], f32, tag=f"ews{d}")
                ew_tiles[d] = ew_sb
                for p0 in range(0, s.nchunk, GCALL):
                    pn = min(GCALL, s.nchunk - p0)
                    sit = sb.tile([128, GCALL * 8], i16, tag="sit")
                    nc.sync.dma_start(sit[:, :pn * 8],
                                      ins[f"sidx_{d}"][:, p0 * 8:(p0 + pn) * 8])
                    dit = sb.tile([128, GCALL * 8], i16, tag="dit")
                    nc.sync.dma_start(dit[:, :pn * 8],
                                      ins[f"didx_{d}"][:, p0 * 8:(p0 + pn) * 8])
                    g1 = mgp.tile([128, GCALL, D], f32, tag="mg")
                    c0 = p0
                    while c0 < p0 + pn:
                        bk = int(np.searchsorted(s.bank_base, c0, side="right") - 1)
                        bend = (s.bank_base[bk + 1] if bk + 1 < s.nbank
                                else s.nchunk)
                        cn = min(p0 + pn, bend) - c0
                        nrow = min(BANK, s.n_src - bk * BANK)
                        nc.gpsimd.dma_gather(
                            g1[:, c0 - p0:c0 - p0 + cn, :],
                            tcc_out[TSRC[d]][bk * BANK:bk * BANK + nrow, :],
                            sit[:, (c0 - p0) * 8:(c0 - p0 + cn) * 8],
                            cn * 128, cn * 128, D, single_packet=False,
                            queue_num=(c0 // GCALL) % 2)
                        c0 += cn
                    g2 = mgp.tile([128, GCALL, D], f32, tag="mg")
                    nc.gpsimd.dma_gather(
                        g2[:, :pn, :], tcc_in[TDST[d]][:, :],
                        dit[:, :pn * 8],
                        pn * 128, pn * 128, D, single_packet=False,
                        queue_num=2 + (p0 // GCALL) % 2)
                    pre = sb.tile([128, GCALL, D], f32, tag="pre")
                    for hh in range(0, pn, 8):
                        hn = min(8, pn - hh)
                        eft = sb.tile([EF + 1, 8 * 128], f32, tag="eft")
                        nc.sync.dma_start(
                            eft[:, :hn * 128],
                            ins[f"efT_{d}"][:, (p0 + hh) * 128:(p0 + hh + hn) * 128])
                        pe = ps.tile([128, 512], f32, tag="mm")
                        pev = pe[:].rearrange("p (a b) -> p a b", b=D)
                        for j in range(hn):
                            mmul(pev[:, j, :],
                                 eft[:, j * 128:(j + 1) * 128],
                                 w1e[:])
                        nc.vector.tensor_tensor(
                            out=pre[:, hh:hh + hn, :], in0=g1[:, hh:hh + hn, :],
                            in1=pev[:, :hn, :], op=ALU.add)
                    nc.vector.tensor_tensor(out=pre[:, :pn, :], in0=pre[:, :pn, :],
                                            in1=g2[:, :pn, :], op=ALU.add)
                    nc.scalar.activation(pre[:, :pn, :], pre[:, :pn, :], AF.Relu)
                    for hh in range(0, pn, 8):
                        hn = min(8, pn - hh)
                        nc.vector.tensor_tensor(out=pre[:, hh:hh + hn, :],
                                                in0=pre[:, hh:hh + hn, :],
                                                in1=w2b[:, :hn, :], op=ALU.mult)
                    sm = sb.tile([128, GCALL], f32, tag="sm")
                    nc.vector.reduce_sum(sm[:, :pn], pre[:, :pn, :], axis=AX.X)
                    nc.scalar.activation(ew_sb[:, p0:p0 + pn], sm[:, :pn],
                                         AF.Sigmoid, bias=b2b[:])

            # ---------------- P2c: degree reciprocals ----------------
            for d in DIRS:
                s = scheds[d]
                dl, ew_sb = dloc_sb[d], ew_tiles[d]
                for wv0 in range(0, s.nwin, GRP):
                    ng = min(GRP, s.nwin - wv0)
                    nn = ng * WIN
                    n0 = wv0 * WIN
                    pdeg = psd.tile([1, GRP * WIN], f32, tag="deg")
                    for w in range(ng):
                        for j in range(s.cpw):
                            k = (wv0 + w) * s.cpw + j
                            col = int(s.colmap[k])
                            w1t = wtp.tile([128, WIN], f32, tag="w1")
                            nc.vector.tensor_scalar(
                                out=w1t[:], in0=iotaB[:],
                                scalar1=dl[:, col:col + 1],
                                scalar2=ew_sb[:, col:col + 1],
                                op0=ALU.is_equal, op1=ALU.mult)
                            mmul(pdeg[:, w * WIN:(w + 1) * WIN],
                                 ones128c[:], w1t[:],
                                 st=(j == 0), sp=(j == s.cpw - 1))
                    dr = sb.tile([1, GRP * WIN], f32, tag="mrow")
                    nc.scalar.copy(dr[:, :nn], pdeg[:, :nn])
                    nc.vector.tensor_scalar_max(dr[:, :nn], dr[:, :nn], 1.0)
                    nc.vector.reciprocal(dr[:, :nn], dr[:, :nn])
                    nv = min(nn, s.shard - n0)
                    nc.sync.dma_start(recip_d[d][:, n0:n0 + nv], dr[:, :nv])

            # ---------------- P3: layers ----------------
            f32r_ = mybir.dt.float32r
            for l in range(LAYERS):
                for di, d in enumerate(DIRS):
                    i = l * 4 + di
                    s = scheds[d]
                    st, dt_ = SRCT[d], DSTT[d]
                    ssh, dsh = NTYPE[st] // NCORES, NTYPE[dt_] // NCORES
                    dl, ew_sb = dloc_sb[d], ew_tiles[d]

                    # --- msg mlp on own src shard -> node-major msg_in ---
                    mw1 = sb.tile([D, D], f32, tag="mw1")
                    nc.sync.dma_start(mw1[:], ins["mp_msg_w1"][i])
                    mw2 = sb.tile([D, D], f32, tag="mw2")
                    nc.sync.dma_start(mw2[:], ins["mp_msg_w2"][i])
                    mb1 = mpb["mp_msg_b1"][:, i:i + 1]
                    mb2 = mpb["mp_msg_b2"][:, i:i + 1]
                    for g0 in range(0, ssh, 512):
                        gw = min(512, ssh - g0)
                        p1 = ps.tile([128, 512], f32, tag="mm")
                        mmul(p1[:D, :gw], mw1[:], hT[st][:, g0:g0 + gw])
                        r1 = sb.tile([D, 512], f32, tag="r1")
                        nc.scalar.activation(r1[:, :gw], p1[:D, :gw], AF.Relu,
                                             bias=mb1)
                        p2 = ps.tile([128, 512], f32, tag="mm")
                        mmul(p2[:D, :gw], mw2[:], r1[:, :gw])
                        mt = sb.tile([D, 512], f32, tag="h1")
                        nc.vector.tensor_scalar_add(mt[:, :gw], p2[:D, :gw], mb2)
                        for j0 in range(0, gw, 128):
                            jw = min(128, gw - j0)
                            pn_ = pst.tile([128, 128], f32, tag="tr")
                            nc.tensor.transpose(pn_[:jw, :D], mt[:, j0:j0 + jw],
                                                ident[:D, :D])
                            nb = sb.tile([128, D], f32, tag="nb")
                            nc.scalar.copy(nb[:jw, :], pn_[:jw, :D])
                            nc.sync.dma_start(
                                msg_in[(l, d)][g0 + j0:g0 + j0 + jw, :], nb[:jw, :])
                    nc.gpsimd.collective_compute(
                        "AllGather", ALU.bypass, ins=[msg_in[(l, d)][:]],
                        outs=[msg_out[(l, d)][:]], replica_groups=RG)

                    # --- update-phase constants ---
                    gwt = sb.tile([2 * D, D], f32, tag="gwt")
                    nc.sync.dma_start(gwt[:], ins["mp_gate_w"][i])
                    uw1 = sb.tile([2 * D, D], f32, tag="uw1")
                    nc.sync.dma_start(uw1[:], ins["mp_upd_w1"][i])
                    uw2 = sb.tile([D, D], f32, tag="uw2")
                    nc.sync.dma_start(uw2[:], ins["mp_upd_w2"][i])
                    gb = mpb["mp_gate_b"][:, i:i + 1]
                    ub1 = mpb["mp_upd_b1"][:, i:i + 1]
                    ub2 = mpb["mp_upd_b2"][:, i:i + 1]
                    lng = mpb["mp_ln_g"][:, i:i + 1]
                    lnb = mpb["mp_ln_b"][:, i:i + 1]

                    cpb_off = np.concatenate([[0], np.cumsum(s.cpb)])

                    for gi, wv0 in enumerate(range(0, s.nwin, GRP)):
                        ng = min(GRP, s.nwin - wv0)
                        nn0 = ng * WIN
                        n0 = wv0 * WIN
                        nn = min(nn0, dsh - n0)
                        # gather each bank's contiguous group slice of the
                        # bank stream: cols [base + wv0*cpb, base+(wv0+ng)*cpb)
                        gt_b, col0_b = [], []
                        for bk in range(s.nbank):
                            ncols = ng * s.cpb[bk]
                            col0 = int(s.bank_base[bk]) + wv0 * s.cpb[bk]
                            col0_b.append(col0)
                            sit = sb.tile([128, GRP * 3 * 8], i16, tag="sit")
                            nc.sync.dma_start(
                                sit[:, :ncols * 8],
                                ins[f"sidx_{d}"][:, col0 * 8:(col0 + ncols) * 8])
                            g = mgp.tile([128, GRP * 3, D], f32, tag="mg",
                                         bufs=5)
                            nrow = min(BANK, s.n_src - bk * BANK)
                            nc.gpsimd.dma_gather(
                                g[:, :ncols, :],
                                msg_out[(l, d)][bk * BANK:bk * BANK + nrow, :],
                                sit[:, :ncols * 8],
                                ncols * 128, ncols * 128, D, single_packet=False,
                                queue_num=(gi * s.nbank + bk) % 4)
                            gt_b.append(g)
                        pgr = psw.tile([D, GRP * WIN], f32, tag="grp")
                        for w in range(ng):
                            for j in range(s.cpw):
                                bk = s.slot_bank[j]
                                jb = j - int(cpb_off[bk])
                                pos = w * s.cpb[bk] + jb
                                col = col0_b[bk] + pos
                                g = gt_b[bk]
                                wt = wtp.tile([128, WIN], f32, tag="w1")
                                nc.vector.tensor_scalar(
                                    out=wt[:], in0=iotaB[:],
                                    scalar1=dl[:, col:col + 1],
                                    scalar2=ew_sb[:, col:col + 1],
                                    op0=ALU.is_equal, op1=ALU.mult)
                                mmul(pgr[:, w * WIN:(w + 1) * WIN],
                                     g[:, pos, :], wt[:],
                                     st=(j == 0), sp=(j == s.cpw - 1))
                        # combT = [agg/deg ; h]
                        rrow = sb.tile([1, GRP * WIN], f32, tag="rr")
                        nc.sync.dma_start(rrow[:, :nn], recip_d[d][:, n0:n0 + nn])
                        combT = sb.tile([2 * D, 512], f32, tag="combT")
                        nc.scalar.copy(combT[0:D, :nn], pgr[:, :nn])
                        pb = psd.tile([D, 512], f32, tag="bc")
                        mmul(pb[:D, :nn], onesD[:], rrow[:, :nn])
                        nc.vector.tensor_tensor(
                            out=combT[0:D, :nn], in0=combT[0:D, :nn],
                            in1=pb[:D, :nn], op=ALU.mult)
                        nc.sync.dma_start(combT[D:2 * D, :nn], hT[dt_][:, n0:n0 + nn])
                        pg = ps.tile([128, 512], f32, tag="mm")
                        mmul(pg[:D, :nn], gwt[:], combT[:, :nn])
                        gt = sb.tile([D, 512], f32, tag="gt")
                        nc.scalar.activation(gt[:, :nn], pg[:D, :nn],
                                             AF.Sigmoid, bias=gb)
                        pu = ps.tile([128, 512], f32, tag="mm")
                        mmul(pu[:D, :nn], uw1[:], combT[:, :nn])
                        ru = sb.tile([D, 512], f32, tag="r1")
                        nc.scalar.activation(ru[:, :nn], pu[:D, :nn], AF.Relu,
                                             bias=ub1)
                        pu2 = ps.tile([128, 512], f32, tag="mm")
                        mmul(pu2[:D, :nn], uw2[:], ru[:, :nn])
                        ut = sb.tile([D, 512], f32, tag="h1")
                        nc.vector.tensor_scalar_add(ut[:, :nn], pu2[:D, :nn], ub2)
                        # out = h + g*(u - h)
                        nc.vector.tensor_tensor(out=ut[:, :nn], in0=ut[:, :nn],
                                                in1=hT[dt_][:, n0:n0 + nn],
                                                op=ALU.subtract)
                        nc.vector.tensor_tensor(out=ut[:, :nn], in0=ut[:, :nn],
                                                in1=gt[:, :nn], op=ALU.mult)
                        nc.vector.tensor_tensor(out=ut[:, :nn], in0=ut[:, :nn],
                                                in1=hT[dt_][:, n0:n0 + nn],
                                                op=ALU.add)
                        # LN along features (partition dim) via matmul stats
                        pm = psd.tile([1, 512], f32, tag="stat")
                        mmul(pm[:, :nn], invDc[:], ut[:, :nn])
                        mrow = sb.tile([1, 512], f32, tag="mrow")
                        nc.scalar.copy(mrow[:, :nn], pm[:, :nn])
                        pmb = psd.tile([D, 512], f32, tag="bc")
                        mmul(pmb[:D, :nn], onesD[:], mrow[:, :nn])
                        nc.vector.tensor_tensor(
                            out=ut[:, :nn], in0=ut[:, :nn],
                            in1=pmb[:D, :nn], op=ALU.subtract)
                        sq = sb.tile([D, 512], f32, tag="gt")
                        nc.vector.tensor_tensor(out=sq[:, :nn], in0=ut[:, :nn],
                                                in1=ut[:, :nn], op=ALU.mult)
                        pv = psd.tile([1, 512], f32, tag="stat")
                        mmul(pv[:, :nn], invDc[:], sq[:, :nn])
                        vrow = sb.tile([1, 512], f32, tag="mrow")
                        nc.scalar.copy(vrow[:, :nn], pv[:, :nn])
                        nc.vector.tensor_scalar_add(vrow[:, :nn], vrow[:, :nn],
                                                    float(LN_EPS))
                        nc.scalar.activation(vrow[:, :nn], vrow[:, :nn], AF.Sqrt)
                        nc.vector.reciprocal(vrow[:, :nn], vrow[:, :nn])
                        pvb = psd.tile([D, 512], f32, tag="bc")
                        mmul(pvb[:D, :nn], onesD[:], vrow[:, :nn])
                        nc.vector.tensor_tensor(
                            out=ut[:, :nn], in0=ut[:, :nn],
                            in1=pvb[:D, :nn], op=ALU.mult)
                        nc.vector.tensor_scalar(
                            out=hT[dt_][:, n0:n0 + nn], in0=ut[:, :nn],
                            scalar1=lng, scalar2=lnb,
                            op0=ALU.mult, op1=ALU.add)

            # ---------------- output: h_cut node-major ----------------
            csh = NK // NCORES
            for g0 in range(0, csh, 512):
                gw = min(512, csh - g0)
                for j0 in range(0, gw, 128):
                    jw = min(128, gw - j0)
                    po = pst.tile([128, 128], f32, tag="tr")
                    nc.tensor.transpose(po[:jw, :D], hT["cut"][:, g0 + j0:g0 + j0 + jw],
                                        ident[:D, :D])
                    nb = sb.tile([128, D], f32, tag="nb")
                    nc.scalar.copy(nb[:jw, :], po[:jw, :D])
                    nc.sync.dma_start(out_t[g0 + j0:g0 + j0 + jw, :], nb[:jw, :])

    nc.compile()
    return nc


def make_inputs(inputs, scheds):
    """Build the per-core input maps from the model inputs + schedules."""
    maps = [dict() for _ in range(NCORES)]
    feats = {"var": "variable_features", "cons": "constraint_features",
             "cut": "cut_features"}
    for t in ("var", "cons", "cut"):
        f = np.asarray(inputs[feats[t]], dtype=np.float32)
        sh = NTYPE[t] // NCORES
        w1 = np.concatenate([np.asarray(inputs[f"{t}_w1"]),
                             np.asarray(inputs[f"{t}_b1"])[None, :]], axis=0)
        for c in range(NCORES):
            ft = np.ones((FD[t] + 1, sh), dtype=np.float32)
            ft[:FD[t], :] = f[c * sh:(c + 1) * sh].T
            maps[c][f"featT_{t}"] = ft
            maps[c][f"emb_{t}_w1"] = np.ascontiguousarray(w1, dtype=np.float32)
            maps[c][f"emb_{t}_w2"] = np.asarray(inputs[f"{t}_w2"], dtype=np.float32)
            maps[c][f"emb_{t}_b2"] = np.asarray(
                inputs[f"{t}_b2"], dtype=np.float32).reshape(1, D)
    iota = np.tile(np.arange(WIN, dtype=np.float32)[None, :], (128, 1))
    for c in range(NCORES):
        maps[c]["iotaB"] = iota
    for d in DIRS:
        s = scheds[d]
        for c in range(NCORES):
            maps[c][f"sidx_{d}"] = s.src_idx[c]
            maps[c][f"didx_{d}"] = s.dst_idx[c]
            maps[c][f"efT_{d}"] = s.efp[c]
            maps[c][f"dlocT_{d}"] = s.dlocT[c]
    for et, pre in (("vc", "ewvc"), ("vk", "ewvk")):
        w1 = np.asarray(inputs[f"{pre}_w1"], dtype=np.float32)
        b1 = np.asarray(inputs[f"{pre}_b1"], dtype=np.float32)
        w2 = np.asarray(inputs[f"{pre}_w2"], dtype=np.float32)
        b2 = np.asarray(inputs[f"{pre}_b2"], dtype=np.float32)
        w1e = np.concatenate([w1[2 * D:2 * D + EF], b1[None, :]], axis=0)
        for c in range(NCORES):
            maps[c][f"ew_{et}_w1v"] = np.ascontiguousarray(w1[0:D])
            maps[c][f"ew_{et}_w1o"] = np.ascontiguousarray(w1[D:2 * D])
            maps[c][f"ew_{et}_w1e"] = np.ascontiguousarray(w1e)
            maps[c][f"ew_{et}_w2t"] = np.tile(w2.reshape(1, D), (1, 8)).astype(
                np.float32)
            maps[c][f"ew_{et}_b2"] = b2.reshape(1, 1)
    for nm in ("mp_msg_w1", "mp_msg_w2", "mp_gate_w", "mp_upd_w1", "mp_upd_w2"):
        a = np.asarray(inputs[nm], dtype=np.float32)
        for c in range(NCORES):
            maps[c][nm] = a
    for nm in ("mp_msg_b1", "mp_msg_b2", "mp_gate_b", "mp_upd_b1", "mp_upd_b2",
               "mp_ln_g", "mp_ln_b"):
        a = np.asarray(inputs[nm], dtype=np.float32).reshape(16, 1, D)
        for c in range(NCORES):
            maps[c][nm] = a
    return maps


_CACHE = {}


def kernel(**inputs):
    inputs = {k: np.asarray(v) for k, v in inputs.items()}
    scheds = build_schedules(inputs)
    key = "k"
    if key not in _CACHE:
        _CACHE[key] = build_kernel(scheds)
    nc = _CACHE[key]
    maps = make_inputs(inputs, scheds)
    res = run_bass_kernel_spmd(nc, maps, core_ids=list(range(NCORES)))
    out = np.concatenate([res.results[c]["out_cut"] for c in range(NCORES)], axis=0)
    return out.astype(np.float32)
